# revision 8
# baseline (speedup 1.0000x reference)
"""Trainium2 Bass kernel for nn_HamiltonianVersorNN.

Math: the reference energy reads only blade-0 of the final layer, and the
versor gate h*sigmoid(h[...,0:1]) makes blade-0 evolve as elementwise SiLU.
Backprop therefore collapses exactly to a 2-layer SiLU MLP on blade-0:

    a1 = A x + c1            A  = W1 @ W_in[:, ::32].T          [32, 6]
    a2 = W2 silu(a1) + c2    c1 = W1 @ b_in[::32] + b1[:, 0]
    dx = A.T (W2.T (w3 * silu'(a2)) * silu'(a1))
    out = x + dt * [dx[3:6], -dx[0:3]]

Sharding: pure data parallel over B*S*N positions, 8 cores, 16384
positions/core. On-chip layout packs 4 tokens per 128-partition column
(partition 32*tl + c holds channel c of token 4g+tl) so the 32x32
channel-mix matmuls contract over the full 128 partitions via
block-diagonal stationaries.

Perf design (v4, from 40.9us baseline):
- The PE streams at ~1.2 GHz on this part (512-col matmul ~427ns;
  dep-free spacing histogram shows nothing near the 2.4 GHz rate), so
  matmul passes over the 4096 columns dominate. Five passes are needed:
  l1 (a1 from x), l2x (a2 partial from x), l2w (a2 from w), l3 (v1 from
  d2), l4 (po from g1). The two x-consuming passes run as fp8e4m3
  DoubleRow (x and their stationaries laid out [13, 2, N]: contraction
  split in two k-tiles, 2 cols/cycle) - x only feeds the gradient path
  (the dt*grad term is ~1e-5 of the output scale), so fp8 there is
  harmless. Everything else is fp16: no fp32-HIGH mode anywhere, FWL
  stays available (fp32r movings silently run 4-pass and poison FWL).
- a2 = W2 silu(a1) + c2 is split as (W2/2)a1 + (W2/2)(a1 tanh(a1/2)):
  the (W2/2)a1 term composes with the input layer and streams straight
  from x; the only layer-1 elementwise product is w = a1*tau1 (DVE).
  Tanh and Derivative_silu share one ACT table set
  (derivative_silu_and_others) so there are zero table switches.
- The residual rides the mandatory PSUM->SBUF evacuation (DMA has no
  PSUM route): out = po + x as a DVE tensor_add against a host-prepared
  quadrant-packed fp32 copy of x - full precision, zero extra cost. po
  chunks are quadrant-packed by the l4 matmuls into partition strips
  (512-col chunk h -> partitions 32h..32h+24), halving the evacuation
  free-size on 1024 blocks.
- Queues: x + outputs on Sync (l12 first - it gates the first matmul),
  the merged back-half stationary block on Scalar (one DMA, done before
  the ACT table load), GpSimd entirely unused. Work tiles are allocated
  at uniform width under 6 tags so the Tile context's end-of-kernel
  semaphore drain stays short.
- Pipelining: back half split in two: (d2, l3) issue right after tau(k)
  so d2(k-1) takes the second ACT slot and l3(k-1) does not block the
  w-dependent l2w(k) on the PE queue; (g1, l4, evac, DMA) issue after
  l2w(k). Two small 256-col drain blocks shorten the final serial tail.
  PSUM: a1 double-buffered [2x2 banks], mid arena double-buffered
  [2x2 banks] rotating a2 -> v1 -> po.
"""

import sys

import numpy as np

if "/opt/trn_rl_repo" not in sys.path:
    sys.path.insert(0, "/opt/trn_rl_repo")

import concourse.bass as bass
import concourse.tile as tile
from concourse import mybir

AF = mybir.ActivationFunctionType
F32 = mybir.dt.float32
F16 = mybir.dt.float16
F8 = mybir.dt.float8e4
DR = mybir.MatmulPerfMode.DoubleRow

N_CORES = 8
B, S, N, D = 32, 256, 16, 6
HIDDEN = 32
BLADES = 32
DT = 0.01

TOK_TOTAL = B * S * N          # 131072 positions
TOK_CORE = TOK_TOTAL // N_CORES  # 16384
TPC = 4                        # tokens packed per 128-partition column
GROUPS = TOK_CORE // TPC       # 4096 columns per core
MM = 512                       # matmul free-dim (1 PSUM bank fp32)
BD = 1024                      # max block free-dim (2 PSUM banks)
BLOCKS = [(0, 512), (512, 1024), (1536, 1024), (2560, 1024),
          (3584, 256), (3840, 256)]
OUTW = 512                     # per-block HBM stride of the packed output
NB = len(BLOCKS)

KP = TPC * D                   # 24 partitions of x / out rows
KPI = KP + 1                   # + constant ones row carrying the biases
KT = 13                        # fp8 DoubleRow k-tile height (2*13 >= 26)


def _chunks(wd):
    """512-col matmul chunks of a block: (h, off, width)."""
    out = []
    h = 0
    off = 0
    while off < wd:
        out.append((h, off, min(MM, wd - off)))
        h += 1
        off += MM
    return out


def _build_nc():
    nc = bass.Bass()

    xg8 = nc.dram_tensor("xg8", [KT, 2, GROUPS], F8, kind="ExternalInput")
    xq = nc.dram_tensor("xq", [64, OUTW * NB], F32, kind="ExternalInput")
    l128 = nc.dram_tensor("l128", [KT, 2, 256], F8, kind="ExternalInput")
    cw = nc.dram_tensor("cw", [128, 280], F16, kind="ExternalInput")
    outg = nc.dram_tensor("outg", [64, OUTW * NB], F32, kind="ExternalOutput")

    with tile.TileContext(nc) as tc:
        with (
            tc.tile_pool(name="consts", bufs=1) as consts,
            tc.tile_pool(name="xin", bufs=1) as xin,
            tc.tile_pool(name="work", bufs=3) as work,
            tc.tile_pool(name="psA", bufs=2, space="PSUM") as psA,
            tc.tile_pool(name="psB", bufs=2, space="PSUM") as psB,
        ):
            # l128 gates the very first matmul, so it leads the Sync queue
            # (ahead of the x slabs). The merged back-half stationary block
            # rides Scalar (one DMA, configured before the ACT table load;
            # it is only needed one block in). GpSimd stays fully idle.
            sb_l128 = consts.tile([KT, 2, 256], F8)
            nc.sync.dma_start(out=sb_l128[:], in_=l128[:])
            sb_l1 = sb_l128[:, :, 0:128]
            sb_l2x = sb_l128[:, :, 128:256]
            sb_cw = consts.tile([128, 280], F16)
            nc.scalar.dma_start(out=sb_cw[:], in_=cw[:])
            sb_l2w = sb_cw[:, 0:128]
            sb_l3 = sb_cw[:, 128:256]
            sb_l4 = sb_cw[:, 256:280]

            # x arrives in two slabs (block 0's lands first), the
            # quadrant-packed fp32 residual copy in one.
            W0 = BLOCKS[0][1]
            sb_x8a = xin.tile([KT, 2, W0], F8)
            nc.sync.dma_start(out=sb_x8a[:], in_=xg8[:, :, 0:W0])
            sb_x8b = xin.tile([KT, 2, GROUPS - W0], F8)
            nc.sync.dma_start(out=sb_x8b[:], in_=xg8[:, :, W0:GROUPS])
            sb_xq = xin.tile([64, OUTW * NB], F32)
            nc.sync.dma_start(out=sb_xq[:], in_=xq[:])

            def xslab(bi, lo, hi):
                """SBUF view of x columns [lo, hi) of block bi."""
                c0 = BLOCKS[bi][0]
                if bi == 0:
                    return sb_x8a[:, :, c0 + lo : c0 + hi]
                return sb_x8b[:, :, c0 - W0 + lo : c0 - W0 + hi]

            # Dummy first activation: walrus attaches the ACT table load to
            # the first Activation instruction, which can then carry only a
            # single sync wait. Give it a single-wait warm-up op.
            warm = consts.tile([1, 128], F32)
            nc.vector.memset(warm[:], 0.0)
            nc.scalar.activation(warm[:], warm[:], AF.Derivative_silu)

            def d2_of(st):
                """d2(k-1): issued right after tau(k) so it takes the
                second ACT slot (its input has been ready since the
                previous block)."""
                bi, wd, mid, d1 = st
                d2 = work.tile([128, BD], F16, tag="d2")
                nc.scalar.activation(d2[:, 0:wd], mid[:, 0:wd],
                                     AF.Derivative_silu)
                return d2

            def v1_of(st, d2):
                """v1(k-1) = blockdiag(diag(w3) W2)^T @ d2, over a2."""
                bi, wd, mid, d1 = st
                for h, off, cw_ in _chunks(wd):
                    ms = slice(off, off + cw_)
                    nc.tensor.matmul(mid[:, ms], sb_l3, d2[:, ms],
                                     start=True, stop=True)

            # All evacuated outputs collect in one SBUF arena so the
            # outputs leave in TWO consolidated DMAs (every DMA's DGE
            # semaphore is re-checked by all five engines in the NEFF
            # teardown drain - fewer DMAs = shorter teardown).
            sb_o = consts.tile([64, OUTW * NB], F32)
            OUT_SPLIT = 3          # blocks 0..2 in DMA 1, 3..5 in DMA 2

            def back_late(st):
                """g1 -> po (quadrant-packed) -> evac(+residual)."""
                bi, wd, mid, d1 = st
                g1 = work.tile([128, BD], F16, tag="g1")
                nc.vector.tensor_mul(g1[:, 0:wd], mid[:, 0:wd], d1[:, 0:wd])

                # po = blockdiag(Bout) @ g1, chunk h on partitions
                # 32h..32h+24 (overwrites v1, dead after g1).
                nch = 0
                for h, off, cw_ in _chunks(wd):
                    po = mid[32 * h : 32 * h + KP, 0:cw_]
                    nc.tensor.matmul(po, sb_l4, g1[:, off : off + cw_],
                                     start=True, stop=True)
                    nch += 1

                # Mandatory PSUM->SBUF evacuation doubles as the residual
                # add against the quadrant-packed fp32 x.
                ew = min(wd, MM)
                nc.vector.tensor_add(
                    sb_o[0 : 32 * nch, OUTW * bi : OUTW * bi + ew],
                    mid[0 : 32 * nch, 0:ew],
                    sb_xq[0 : 32 * nch, OUTW * bi : OUTW * bi + ew],
                )
                if bi == OUT_SPLIT - 1:
                    nc.sync.dma_start(
                        out=outg[:, 0 : OUTW * OUT_SPLIT],
                        in_=sb_o[:, 0 : OUTW * OUT_SPLIT],
                    )
                elif bi == NB - 1:
                    nc.sync.dma_start(
                        out=outg[:, OUTW * OUT_SPLIT :],
                        in_=sb_o[:, OUTW * OUT_SPLIT :],
                    )

            def l1_of(bi):
                """a1(bi) = blockdiag(A) @ x + c1 (c1 rides the ones row);
                fp8 DoubleRow, contraction over 2 k-tiles of 13 rows.
                Depends only on the prefetched x, so block bi+1's copy is
                hoisted into block bi to keep the in-order PE queue from
                starving behind the dependency-gated l2w/l3/l4 matmuls
                (its psA banks died with d1(bi-2), so no WAR stall)."""
                wd = BLOCKS[bi][1]
                a1 = psA.tile([128, BD], F32, tag="a1")
                for h, off, cw_ in _chunks(wd):
                    nc.tensor.matmul(a1[:, off : off + cw_], sb_l1,
                                     xslab(bi, off, off + cw_),
                                     start=True, stop=True, perf_mode=DR)
                return a1

            def l2x_of(bi):
                """The x-borne half of a2(bi): a2 = W2 silu(a1) + c2 is
                split as (W2/2)a1 + (W2/2)(a1*tau); the first term composes
                with the input layer (l2x = blockdiag(W2 A / 2), bias row
                W2 c1 / 2 + c2) and streams straight from x. Issued right
                after evac(bi-2) - its psB banks are freed by that evac,
                so issuing it any earlier would head-of-line block the PE
                queue on the WAR wait."""
                wd = BLOCKS[bi][1]
                mid = psB.tile([128, BD], F32, tag="mid")
                for h, off, cw_ in _chunks(wd):
                    nc.tensor.matmul(mid[:, off : off + cw_], sb_l2x,
                                     xslab(bi, off, off + cw_),
                                     start=True, stop=False, perf_mode=DR)
                return mid

            a1 = l1_of(0)
            mid = l2x_of(0)
            a1n = mid_n = None
            pending = None
            for bi, (c0, wd) in enumerate(BLOCKS):
                if bi > 0:
                    a1, mid = a1n, mid_n

                # tau = tanh(0.5*a1)
                tau = work.tile([128, BD], F16, tag="tau")
                nc.scalar.activation(tau[:, 0:wd], a1[:, 0:wd], AF.Tanh,
                                     scale=0.5)

                # w = a1 * tau; issued now so it leads the DVE queue.
                w = work.tile([128, BD], F16, tag="w")
                nc.vector.tensor_mul(w[:, 0:wd], a1[:, 0:wd], tau[:, 0:wd])

                # d2(k-1) takes the second ACT slot.
                d2p = d2_of(pending) if pending is not None else None

                # d1 = silu'(a1) (consumer g1 is a block away)
                d1 = work.tile([128, BD], F16, tag="d1")
                nc.scalar.activation(d1[:, 0:wd], a1[:, 0:wd],
                                     AF.Derivative_silu)

                # PE queue, in expected-readiness order: l1(k+1) (ready
                # now), l3(k-1) (after the d2 above), l2w(k) (after w),
                # then in back_late l4(k-1) (after g1) and l2x(k+1)
                # (after evac(k-1), issued there).
                if bi + 1 < NB:
                    a1n = l1_of(bi + 1)
                if pending is not None:
                    v1_of(pending, d2p)

                # a2 += blockdiag(W2/2) @ w
                for h, off, cw_ in _chunks(wd):
                    ms = slice(off, off + cw_)
                    nc.tensor.matmul(mid[:, ms], sb_l2w, w[:, ms],
                                     start=False, stop=True)

                if pending is not None:
                    back_late(pending)
                if bi + 1 < NB:
                    mid_n = l2x_of(bi + 1)

                pending = (bi, wd, mid, d1)

            d2p = d2_of(pending)
            v1_of(pending, d2p)
            back_late(pending)

    return nc


def _split_multi_waits(nc):
    """This walrus build rejects engine instructions carrying more than one
    sync wait ("Too many sync wait commands"). Hoist all but one wait of
    each instruction onto standalone NoOps issued just before it on the
    same engine (engines execute their queue in order, so semantics are
    preserved)."""
    for f in nc.m.functions:
        for b in f.blocks:
            insts = list(b.instructions)
            out = []
            changed = False
            for inst in insts:
                # This walrus build also rejects the raw-ISA
                # EVENT_SEMAPHORE_RANGE_CLEAR Tile emits at context end
                # ("ISA wrong length" - ISA table version skew). The NEFF
                # preamble re-initializes semaphores, so drop it.
                if (
                    type(inst).__name__ == "InstISA"
                    and getattr(inst, "op_name", "") == "EVENT_SEMAPHORE_RANGE_CLEAR"
                ):
                    changed = True
                    continue
                si = getattr(inst, "sync_info", None)
                waits = list(si.on_wait) if si is not None and si.on_wait else []
                if len(waits) > 1:
                    changed = True
                    for k, w in enumerate(waits[:-1]):
                        nop = mybir.InstNoOp(name=f"{inst.name}-w{k}", ins=[], outs=[])
                        nop.engine = inst.engine
                        nop.sync_info = mybir.SyncInfo(on_wait=[w], on_update=[])
                        out.append(nop)
                    inst.sync_info = mybir.SyncInfo(
                        on_wait=[waits[-1]], on_update=list(si.on_update or [])
                    )
                out.append(inst)
            if changed:
                b.instructions = out
    return nc


_NC_CACHE = None


def _get_nc():
    global _NC_CACHE
    if _NC_CACHE is None:
        _NC_CACHE = _split_multi_waits(_build_nc())
    return _NC_CACHE


def _fold_kt(rows):
    """[26, M] -> [13, 2, M] DoubleRow k-tile layout (row = 13*ko + ki)."""
    return rows.reshape(2, KT, -1).transpose(1, 0, 2)


def _prep_weights(W_in, b_in, W1, b1, W2, b2, W3, b3):
    """Host-side constant folding into the kernel's stationary layouts."""
    import ml_dtypes

    F8NP = ml_dtypes.float8_e4m3

    W_in = np.asarray(W_in, np.float64)
    b_in = np.asarray(b_in, np.float64)
    W1 = np.asarray(W1, np.float64)
    b1 = np.asarray(b1, np.float64)
    W2 = np.asarray(W2, np.float64)
    b2 = np.asarray(b2, np.float64)
    W3 = np.asarray(W3, np.float64)

    Win0 = W_in[:, ::BLADES]            # [6, 8]
    bin0 = b_in[::BLADES]               # [8]
    A = W1 @ Win0.T                     # [32, 6]
    c1 = W1 @ bin0 + b1[:, 0]           # [32]
    c2 = b2[:, 0]                       # [32]
    w3 = W3[0, :]                       # [32]

    # Bout[d, c]: out[d] += dt*dx[d+3] (d<3), -dt*dx[d-3] (d>=3); dx = A^T g1
    Bout = np.zeros((D, HIDDEN))
    Bout[0:3, :] = DT * A[:, 3:6].T
    Bout[3:6, :] = -DT * A[:, 0:3].T

    # a2 = W2 silu(a1) + c2 = (W2/2) a1 + (W2/2)(a1 tau1) + c2, and
    # (W2/2) a1 = (W2 A / 2) x + W2 c1 / 2  composes with the input layer.
    A2x = 0.5 * W2 @ A                  # [32, 6]
    c2x = 0.5 * W2 @ c1 + c2            # [32]

    l12 = np.zeros((2 * KT, 256), np.float32)
    l1 = l12[:, 0:128]
    l2xm = l12[:, 128:256]
    cwm = np.zeros((128, 280), np.float16)
    l2wm = cwm[:, 0:128]
    l3 = cwm[:, 128:256]
    l4 = cwm[:, 256:280]
    for tl in range(TPC):
        # l1[6tl+d, 32tl+c] = A[c, d]; l1[24, 32tl+c] = c1[c]
        l1[6 * tl : 6 * tl + 6, 32 * tl : 32 * tl + 32] = A.T
        l1[KP, 32 * tl : 32 * tl + 32] = c1
        # l2x[6tl+d, 32tl+c] = A2x[c, d]; ones row carries c2x
        l2xm[6 * tl : 6 * tl + 6, 32 * tl : 32 * tl + 32] = A2x.T
        l2xm[KP, 32 * tl : 32 * tl + 32] = c2x
        # l2w[32tl+ci, 32tl+co] = W2[co, ci] / 2
        l2wm[32 * tl : 32 * tl + 32, 32 * tl : 32 * tl + 32] = (
            0.5 * W2.T
        ).astype(np.float16)
        # l3[32tl+co, 32tl+ci] = w3[co] * W2[co, ci]
        l3[32 * tl : 32 * tl + 32, 32 * tl : 32 * tl + 32] = (
            w3[:, None] * W2
        ).astype(np.float16)
        # l4[32tl+c, 6tl+d] = Bout[d, c]
        l4[32 * tl : 32 * tl + 32, 6 * tl : 6 * tl + 6] = Bout.T.astype(
            np.float16
        )

    return {
        "l128": np.ascontiguousarray(_fold_kt(l12)).astype(F8NP),
        "cw": cwm,
    }


def _shard_x(x):
    """[B,S,N,D] -> per-core (fp8 [13, 2, GROUPS] DoubleRow matmul layout
    with ones row, fp32 [64, OUTW*NB] quadrant-packed residual layout)."""
    import ml_dtypes

    F8NP = ml_dtypes.float8_e4m3

    xf = np.ascontiguousarray(np.asarray(x, np.float32)).reshape(TOK_TOTAL, D)
    shards = []
    for c in range(N_CORES):
        xc = xf[c * TOK_CORE : (c + 1) * TOK_CORE]          # [16384, 6]
        xp = xc.reshape(GROUPS, TPC, D).transpose(1, 2, 0).reshape(KP, GROUPS)
        xe = np.zeros((2 * KT, GROUPS), np.float32)
        xe[:KP] = xp
        xe[KP] = 1.0
        x8 = np.ascontiguousarray(_fold_kt(xe)).astype(F8NP)
        xqc = np.zeros((64, OUTW * NB), np.float32)
        for bi, (c0, wd) in enumerate(BLOCKS):
            for h, off, cw_ in _chunks(wd):
                xqc[32 * h : 32 * h + KP, OUTW * bi : OUTW * bi + cw_] = xp[
                    :, c0 + off : c0 + off + cw_
                ]
        shards.append((x8, xqc))
    return shards


def _unshard_out(outs):
    """list of per-core [64, OUTW*NB] -> [B,S,N,D].

    Block bi covers global cols c0..c0+wd; its 512-col chunk h sits on
    partitions 32h..32h+24 of outg[:, OUTW*bi : OUTW*bi+cw]."""
    full = np.empty((TOK_TOTAL, D), np.float32)
    for c, og in enumerate(outs):
        og = np.asarray(og)
        oc = np.empty((KP, GROUPS), np.float32)
        for bi, (c0, wd) in enumerate(BLOCKS):
            for h, off, cw_ in _chunks(wd):
                oc[:, c0 + off : c0 + off + cw_] = og[
                    32 * h : 32 * h + KP, OUTW * bi : OUTW * bi + cw_
                ]
        occ = oc.reshape(TPC, D, GROUPS).transpose(2, 0, 1).reshape(TOK_CORE, D)
        full[c * TOK_CORE : (c + 1) * TOK_CORE] = occ
    return full.reshape(B, S, N, D)


# Test-harness knobs (ignored in normal use): set kernel._TRACE = True to
# collect an NTFF profile; the BassKernelResults lands in kernel._LAST_RES.
_TRACE = False
_LAST_RES = None


def kernel(x, W_in, b_in, W1, b1, W2, b2, W3, b3):
    global _LAST_RES
    from concourse.bass_utils import run_bass_kernel_spmd

    nc = _get_nc()
    consts = _prep_weights(W_in, b_in, W1, b1, W2, b2, W3, b3)
    shards = _shard_x(x)
    in_maps = [
        {"xg8": shards[c][0], "xq": shards[c][1], **consts}
        for c in range(N_CORES)
    ]
    res = run_bass_kernel_spmd(nc, in_maps, list(range(N_CORES)), trace=_TRACE)
    _LAST_RES = res
    return _unshard_out([res.results[c]["outg"] for c in range(N_CORES)])


# revision 14
# speedup vs baseline: 1.0605x; 1.0605x over previous
"""Trainium2 Bass kernel for nn_HamiltonianVersorNN.

Math: the reference energy reads only blade-0 of the final layer, and the
versor gate h*sigmoid(h[...,0:1]) makes blade-0 evolve as elementwise SiLU.
Backprop therefore collapses exactly to a 2-layer SiLU MLP on blade-0:

    a1 = A x + c1            A  = W1 @ W_in[:, ::32].T          [32, 6]
    a2 = W2 silu(a1) + c2    c1 = W1 @ b_in[::32] + b1[:, 0]
    dx = A.T (W2.T (w3 * silu'(a2)) * silu'(a1))
    out = x + dt * [dx[3:6], -dx[0:3]]

Sharding: pure data parallel over B*S*N positions, 8 cores, 16384
positions/core. On-chip layout packs 4 tokens per 128-partition column
(partition 32*tl + c holds channel c of token 4g+tl) so the 32x32
channel-mix matmuls contract over the full 128 partitions via
block-diagonal stationaries.

Perf design (v4, from 40.9us baseline):
- The PE streams at ~1.2 GHz on this part (512-col matmul ~427ns;
  dep-free spacing histogram shows nothing near the 2.4 GHz rate), so
  matmul passes over the 4096 columns dominate. Five passes are needed:
  l1 (a1 from x), l2x (a2 partial from x), l2w (a2 from w), l3 (v1 from
  d2), l4 (po from g1). The two x-consuming passes run as fp8e4m3
  DoubleRow (x and their stationaries laid out [13, 2, N]: contraction
  split in two k-tiles, 2 cols/cycle) - x only feeds the gradient path
  (the dt*grad term is ~1e-5 of the output scale), so fp8 there is
  harmless. Everything else is fp16: no fp32-HIGH mode anywhere, FWL
  stays available (fp32r movings silently run 4-pass and poison FWL).
- a2 = W2 silu(a1) + c2 is split as (W2/2)a1 + (W2/2)(a1 tanh(a1/2)):
  the (W2/2)a1 term composes with the input layer and streams straight
  from x; the only layer-1 elementwise product is w = a1*tau1 (DVE).
  Tanh and Derivative_silu share one ACT table set
  (derivative_silu_and_others) so there are zero table switches.
- The residual rides the mandatory PSUM->SBUF evacuation (DMA has no
  PSUM route): out = po + x as a DVE tensor_add against a host-prepared
  quadrant-packed fp32 copy of x - full precision, zero extra cost. po
  chunks are quadrant-packed by the l4 matmuls into partition strips
  (512-col chunk h -> partitions 32h..32h+24), halving the evacuation
  free-size on 1024 blocks.
- Queues: x + outputs on Sync (l12 first - it gates the first matmul),
  the merged back-half stationary block on Scalar (one DMA, done before
  the ACT table load), GpSimd entirely unused. Work tiles are allocated
  at uniform width under 6 tags so the Tile context's end-of-kernel
  semaphore drain stays short.
- Pipelining: back half split in two: (d2, l3) issue right after tau(k)
  so d2(k-1) takes the second ACT slot and l3(k-1) does not block the
  w-dependent l2w(k) on the PE queue; (g1, l4, evac, DMA) issue after
  l2w(k). Two small 256-col drain blocks shorten the final serial tail.
  PSUM: a1 double-buffered [2x2 banks], mid arena double-buffered
  [2x2 banks] rotating a2 -> v1 -> po.
"""

import sys

import numpy as np

if "/opt/trn_rl_repo" not in sys.path:
    sys.path.insert(0, "/opt/trn_rl_repo")

import concourse.bass as bass
import concourse.tile as tile
from concourse import mybir

AF = mybir.ActivationFunctionType
F32 = mybir.dt.float32
F16 = mybir.dt.float16
F8 = mybir.dt.float8e4
DR = mybir.MatmulPerfMode.DoubleRow

N_CORES = 8
B, S, N, D = 32, 256, 16, 6
HIDDEN = 32
BLADES = 32
DT = 0.01

TOK_TOTAL = B * S * N          # 131072 positions
TOK_CORE = TOK_TOTAL // N_CORES  # 16384
TPC = 4                        # tokens packed per 128-partition column
GROUPS = TOK_CORE // TPC       # 4096 columns per core
MM = 512                       # matmul free-dim (1 PSUM bank fp32)
BD = 1024                      # max block free-dim (2 PSUM banks)
BLOCKS = [(0, 512), (512, 1024), (1536, 1024), (2560, 1024),
          (3584, 256), (3840, 256)]
OUTW = 512                     # per-block HBM stride of the packed output
NB = len(BLOCKS)

KP = TPC * D                   # 24 partitions of x / out rows
KPI = KP + 1                   # + constant ones row carrying the biases
KT = 13                        # fp8 DoubleRow k-tile height (2*13 >= 26)


def _chunks(wd):
    """512-col matmul chunks of a block: (h, off, width)."""
    out = []
    h = 0
    off = 0
    while off < wd:
        out.append((h, off, min(MM, wd - off)))
        h += 1
        off += MM
    return out


def _build_nc():
    nc = bass.Bass()

    xg8 = nc.dram_tensor("xg8", [KT, 2, GROUPS], F8, kind="ExternalInput")
    xq = nc.dram_tensor("xq", [64, OUTW * NB], F32, kind="ExternalInput")
    l128 = nc.dram_tensor("l128", [KT, 2, 128], F8, kind="ExternalInput")
    cw = nc.dram_tensor("cw", [128, 280], F16, kind="ExternalInput")
    c2b = nc.dram_tensor("c2b", [128, 1], F32, kind="ExternalInput")
    outg = nc.dram_tensor("outg", [64, OUTW * NB], F32, kind="ExternalOutput")

    with tile.TileContext(nc) as tc:
        with (
            tc.tile_pool(name="consts", bufs=1) as consts,
            tc.tile_pool(name="xin", bufs=1) as xin,
            tc.tile_pool(name="work", bufs=3) as work,
            tc.tile_pool(name="psA", bufs=2, space="PSUM") as psA,
            tc.tile_pool(name="psB", bufs=2, space="PSUM") as psB,
        ):
            # l128 gates the very first matmul, so it leads the Sync queue
            # (ahead of the x slabs). The merged back-half stationary block
            # rides Scalar (one DMA, configured before the ACT table load;
            # it is only needed one block in). GpSimd stays fully idle.
            sb_l128 = consts.tile([KT, 2, 128], F8)
            nc.sync.dma_start(out=sb_l128[:], in_=l128[:])
            sb_l1 = sb_l128[:, :, 0:128]
            sb_cw = consts.tile([128, 280], F16)
            nc.scalar.dma_start(out=sb_cw[:], in_=cw[:])
            sb_l2w = sb_cw[:, 0:128]
            sb_l3 = sb_cw[:, 128:256]
            sb_l4 = sb_cw[:, 256:280]
            sb_c2b = consts.tile([128, 1], F32)
            nc.scalar.dma_start(out=sb_c2b[:], in_=c2b[:])

            # x arrives in two slabs (block 0's lands first), the
            # quadrant-packed fp32 residual copy in one.
            W0 = BLOCKS[0][1]
            sb_x8a = xin.tile([KT, 2, W0], F8)
            nc.sync.dma_start(out=sb_x8a[:], in_=xg8[:, :, 0:W0])
            sb_x8b = xin.tile([KT, 2, GROUPS - W0], F8)
            nc.sync.dma_start(out=sb_x8b[:], in_=xg8[:, :, W0:GROUPS])
            sb_xq = xin.tile([64, OUTW * NB], F32)
            nc.sync.dma_start(out=sb_xq[:], in_=xq[:])

            def xslab(bi, lo, hi):
                """SBUF view of x columns [lo, hi) of block bi."""
                c0 = BLOCKS[bi][0]
                if bi == 0:
                    return sb_x8a[:, :, c0 + lo : c0 + hi]
                return sb_x8b[:, :, c0 - W0 + lo : c0 - W0 + hi]

            # Dummy first activation: walrus attaches the ACT table load to
            # the first Activation instruction, which can then carry only a
            # single sync wait. Give it a single-wait warm-up op.
            warm = consts.tile([1, 128], F32)
            nc.vector.memset(warm[:], 0.0)
            nc.scalar.activation(warm[:], warm[:], AF.Derivative_silu)

            def back_early(st):
                """d2 -> v1 for the previous block. Issued right after
                tau(k) so d2(k-1) - whose input has been ready since last
                block - takes the second ACT slot, and l3(k-1) sits ahead
                of the s'-dependent l2(k) on the PE queue. The c2 bias
                rides the ACTIVATE's per-partition bias port (mid holds
                the unbiased a2)."""
                bi, wd, mid, d1 = st
                d2 = work.tile([128, BD], F16, tag="d2")
                nc.scalar.activation(d2[:, 0:wd], mid[:, 0:wd],
                                     AF.Derivative_silu, bias=sb_c2b[:, 0:1])
                for h, off, cw_ in _chunks(wd):
                    ms = slice(off, off + cw_)
                    nc.tensor.matmul(mid[:, ms], sb_l3, d2[:, ms],
                                     start=True, stop=True)
                return d2

            def back_late(st):
                """g1 -> po (quadrant-packed) -> evac(+residual) -> DMA."""
                bi, wd, mid, d1 = st
                g1 = work.tile([128, BD], F16, tag="g1")
                nc.vector.tensor_mul(g1[:, 0:wd], mid[:, 0:wd], d1[:, 0:wd])

                # po = blockdiag(Bout) @ g1, chunk h on partitions
                # 32h..32h+24 (overwrites v1, dead after g1).
                nch = 0
                for h, off, cw_ in _chunks(wd):
                    po = mid[32 * h : 32 * h + KP, 0:cw_]
                    nc.tensor.matmul(po, sb_l4, g1[:, off : off + cw_],
                                     start=True, stop=True)
                    nch += 1

                # Mandatory PSUM->SBUF evacuation doubles as the residual
                # add against the quadrant-packed fp32 x.
                ew = min(wd, MM)
                sb_o = work.tile([64, MM], F32, tag="o")
                nc.vector.tensor_add(
                    sb_o[0 : 32 * nch, 0:ew],
                    mid[0 : 32 * nch, 0:ew],
                    sb_xq[0 : 32 * nch, OUTW * bi : OUTW * bi + ew],
                )
                nc.sync.dma_start(
                    out=outg[0 : 32 * nch, OUTW * bi : OUTW * bi + ew],
                    in_=sb_o[0 : 32 * nch, 0:ew],
                )

            pending = None
            for bi, (c0, wd) in enumerate(BLOCKS):
                # a1 = blockdiag(A) @ x + c1 (c1 rides the ones row);
                # fp8 DoubleRow: contraction over 2 k-tiles of 13 rows.
                a1 = psA.tile([128, BD], F32, tag="a1")
                for h, off, cw_ in _chunks(wd):
                    nc.tensor.matmul(a1[:, off : off + cw_], sb_l1,
                                     xslab(bi, off, off + cw_),
                                     start=True, stop=True, perf_mode=DR)

                # tau = tanh(0.5*a1)
                tau = work.tile([128, BD], F16, tag="tau")
                nc.scalar.activation(tau[:, 0:wd], a1[:, 0:wd], AF.Tanh,
                                     scale=0.5)

                # s' = (tau + 1) * a1 = 2*silu(a1), one fused DVE op
                # (scalar_tensor_tensor: out = (in0 op0 scalar) op1 in1).
                # This folds what used to be a separate x-borne matmul pass
                # ((W2/2) a1 streamed from x) into the existing elementwise
                # slot: a2 = (W2/2) s' in a single PE pass.
                w = work.tile([128, BD], F16, tag="w")
                nc.vector.scalar_tensor_tensor(
                    w[:, 0:wd], tau[:, 0:wd], 1.0, a1[:, 0:wd],
                    op0=mybir.AluOpType.add, op1=mybir.AluOpType.mult,
                )

                if pending is not None:
                    back_early(pending)

                # d1 = silu'(a1) (consumer g1 is a block away, so d2(k-1)
                # above takes the second ACT slot)
                d1 = work.tile([128, BD], F16, tag="d1")
                nc.scalar.activation(d1[:, 0:wd], a1[:, 0:wd],
                                     AF.Derivative_silu)

                # a2 (sans c2, added at the d2 ACTIVATE) = blockdiag(W2/2) @ s'
                mid = psB.tile([128, BD], F32, tag="mid")
                for h, off, cw_ in _chunks(wd):
                    ms = slice(off, off + cw_)
                    nc.tensor.matmul(mid[:, ms], sb_l2w, w[:, ms],
                                     start=True, stop=True)

                if pending is not None:
                    back_late(pending)

                pending = (bi, wd, mid, d1)

            back_early(pending)
            back_late(pending)

    return nc


def _split_multi_waits(nc):
    """This walrus build rejects engine instructions carrying more than one
    sync wait ("Too many sync wait commands"). Hoist all but one wait of
    each instruction onto standalone NoOps issued just before it on the
    same engine (engines execute their queue in order, so semantics are
    preserved)."""
    for f in nc.m.functions:
        for b in f.blocks:
            insts = list(b.instructions)
            out = []
            changed = False
            for inst in insts:
                # This walrus build also rejects the raw-ISA
                # EVENT_SEMAPHORE_RANGE_CLEAR Tile emits at context end
                # ("ISA wrong length" - ISA table version skew). The NEFF
                # preamble re-initializes semaphores, so drop it.
                if (
                    type(inst).__name__ == "InstISA"
                    and getattr(inst, "op_name", "") == "EVENT_SEMAPHORE_RANGE_CLEAR"
                ):
                    changed = True
                    continue
                si = getattr(inst, "sync_info", None)
                waits = list(si.on_wait) if si is not None and si.on_wait else []
                if len(waits) > 1:
                    changed = True
                    for k, w in enumerate(waits[:-1]):
                        nop = mybir.InstNoOp(name=f"{inst.name}-w{k}", ins=[], outs=[])
                        nop.engine = inst.engine
                        nop.sync_info = mybir.SyncInfo(on_wait=[w], on_update=[])
                        out.append(nop)
                    inst.sync_info = mybir.SyncInfo(
                        on_wait=[waits[-1]], on_update=list(si.on_update or [])
                    )
                out.append(inst)
            if changed:
                b.instructions = out
    return nc


_NC_CACHE = None


def _get_nc():
    global _NC_CACHE
    if _NC_CACHE is None:
        _NC_CACHE = _split_multi_waits(_build_nc())
    return _NC_CACHE


def _fold_kt(rows):
    """[26, M] -> [13, 2, M] DoubleRow k-tile layout (row = 13*ko + ki)."""
    return rows.reshape(2, KT, -1).transpose(1, 0, 2)


def _prep_weights(W_in, b_in, W1, b1, W2, b2, W3, b3):
    """Host-side constant folding into the kernel's stationary layouts."""
    import ml_dtypes

    F8NP = ml_dtypes.float8_e4m3

    W_in = np.asarray(W_in, np.float64)
    b_in = np.asarray(b_in, np.float64)
    W1 = np.asarray(W1, np.float64)
    b1 = np.asarray(b1, np.float64)
    W2 = np.asarray(W2, np.float64)
    b2 = np.asarray(b2, np.float64)
    W3 = np.asarray(W3, np.float64)

    Win0 = W_in[:, ::BLADES]            # [6, 8]
    bin0 = b_in[::BLADES]               # [8]
    A = W1 @ Win0.T                     # [32, 6]
    c1 = W1 @ bin0 + b1[:, 0]           # [32]
    c2 = b2[:, 0]                       # [32]
    w3 = W3[0, :]                       # [32]

    # Bout[d, c]: out[d] += dt*dx[d+3] (d<3), -dt*dx[d-3] (d>=3); dx = A^T g1
    Bout = np.zeros((D, HIDDEN))
    Bout[0:3, :] = DT * A[:, 3:6].T
    Bout[3:6, :] = -DT * A[:, 0:3].T

    # a2 = W2 silu(a1) + c2 = (W2/2) s' + c2 with s' = (1 + tanh(a1/2)) a1
    # = 2 silu(a1); c2 is applied through the d2 ACTIVATE's bias port.
    l1f = np.zeros((2 * KT, 128), np.float32)
    cwm = np.zeros((128, 280), np.float16)
    l2wm = cwm[:, 0:128]
    l3 = cwm[:, 128:256]
    l4 = cwm[:, 256:280]
    c2b = np.zeros((128, 1), np.float32)
    for tl in range(TPC):
        # l1[6tl+d, 32tl+c] = A[c, d]; l1[24, 32tl+c] = c1[c]
        l1f[6 * tl : 6 * tl + 6, 32 * tl : 32 * tl + 32] = A.T
        l1f[KP, 32 * tl : 32 * tl + 32] = c1
        # l2w[32tl+ci, 32tl+co] = W2[co, ci] / 2
        l2wm[32 * tl : 32 * tl + 32, 32 * tl : 32 * tl + 32] = (
            0.5 * W2.T
        ).astype(np.float16)
        # l3[32tl+co, 32tl+ci] = w3[co] * W2[co, ci]
        l3[32 * tl : 32 * tl + 32, 32 * tl : 32 * tl + 32] = (
            w3[:, None] * W2
        ).astype(np.float16)
        # l4[32tl+c, 6tl+d] = Bout[d, c]
        l4[32 * tl : 32 * tl + 32, 6 * tl : 6 * tl + 6] = Bout.T.astype(
            np.float16
        )
        c2b[32 * tl : 32 * tl + 32, 0] = c2

    return {
        "l128": np.ascontiguousarray(_fold_kt(l1f)).astype(F8NP),
        "cw": cwm,
        "c2b": c2b,
    }


def _shard_x(x):
    """[B,S,N,D] -> per-core (fp8 [13, 2, GROUPS] DoubleRow matmul layout
    with ones row, fp32 [64, OUTW*NB] quadrant-packed residual layout)."""
    import ml_dtypes

    F8NP = ml_dtypes.float8_e4m3

    xf = np.ascontiguousarray(np.asarray(x, np.float32)).reshape(TOK_TOTAL, D)
    shards = []
    for c in range(N_CORES):
        xc = xf[c * TOK_CORE : (c + 1) * TOK_CORE]          # [16384, 6]
        xp = xc.reshape(GROUPS, TPC, D).transpose(1, 2, 0).reshape(KP, GROUPS)
        xe = np.zeros((2 * KT, GROUPS), np.float32)
        xe[:KP] = xp
        xe[KP] = 1.0
        x8 = np.ascontiguousarray(_fold_kt(xe)).astype(F8NP)
        xqc = np.zeros((64, OUTW * NB), np.float32)
        for bi, (c0, wd) in enumerate(BLOCKS):
            for h, off, cw_ in _chunks(wd):
                xqc[32 * h : 32 * h + KP, OUTW * bi : OUTW * bi + cw_] = xp[
                    :, c0 + off : c0 + off + cw_
                ]
        shards.append((x8, xqc))
    return shards


def _unshard_out(outs):
    """list of per-core [64, OUTW*NB] -> [B,S,N,D].

    Block bi covers global cols c0..c0+wd; its 512-col chunk h sits on
    partitions 32h..32h+24 of outg[:, OUTW*bi : OUTW*bi+cw]."""
    full = np.empty((TOK_TOTAL, D), np.float32)
    for c, og in enumerate(outs):
        og = np.asarray(og)
        oc = np.empty((KP, GROUPS), np.float32)
        for bi, (c0, wd) in enumerate(BLOCKS):
            for h, off, cw_ in _chunks(wd):
                oc[:, c0 + off : c0 + off + cw_] = og[
                    32 * h : 32 * h + KP, OUTW * bi : OUTW * bi + cw_
                ]
        occ = oc.reshape(TPC, D, GROUPS).transpose(2, 0, 1).reshape(TOK_CORE, D)
        full[c * TOK_CORE : (c + 1) * TOK_CORE] = occ
    return full.reshape(B, S, N, D)


# Test-harness knobs (ignored in normal use): set kernel._TRACE = True to
# collect an NTFF profile; the BassKernelResults lands in kernel._LAST_RES.
_TRACE = False
_LAST_RES = None


def kernel(x, W_in, b_in, W1, b1, W2, b2, W3, b3):
    global _LAST_RES
    from concourse.bass_utils import run_bass_kernel_spmd

    nc = _get_nc()
    consts = _prep_weights(W_in, b_in, W1, b1, W2, b2, W3, b3)
    shards = _shard_x(x)
    in_maps = [
        {"xg8": shards[c][0], "xq": shards[c][1], **consts}
        for c in range(N_CORES)
    ]
    res = run_bass_kernel_spmd(nc, in_maps, list(range(N_CORES)), trace=_TRACE)
    _LAST_RES = res
    return _unshard_out([res.results[c]["outg"] for c in range(N_CORES)])


# revision 15
# speedup vs baseline: 1.1268x; 1.0625x over previous
"""Trainium2 Bass kernel for nn_HamiltonianVersorNN.

Math: the reference energy reads only blade-0 of the final layer, and the
versor gate h*sigmoid(h[...,0:1]) makes blade-0 evolve as elementwise SiLU.
Backprop therefore collapses exactly to a 2-layer SiLU MLP on blade-0:

    a1 = A x + c1            A  = W1 @ W_in[:, ::32].T          [32, 6]
    a2 = W2 silu(a1) + c2    c1 = W1 @ b_in[::32] + b1[:, 0]
    dx = A.T (W2.T (w3 * silu'(a2)) * silu'(a1))
    out = x + dt * [dx[3:6], -dx[0:3]]

Sharding: pure data parallel over B*S*N positions, 8 cores, 16384
positions/core. On-chip layout packs 4 tokens per 128-partition column
(partition 32*tl + c holds channel c of token 4g+tl) so the 32x32
channel-mix matmuls contract over the full 128 partitions via
block-diagonal stationaries.

Perf design (v4, from 40.9us baseline):
- The PE streams at ~1.2 GHz on this part (512-col matmul ~427ns;
  dep-free spacing histogram shows nothing near the 2.4 GHz rate), so
  matmul passes over the 4096 columns dominate. Five passes are needed:
  l1 (a1 from x), l2x (a2 partial from x), l2w (a2 from w), l3 (v1 from
  d2), l4 (po from g1). The two x-consuming passes run as fp8e4m3
  DoubleRow (x and their stationaries laid out [13, 2, N]: contraction
  split in two k-tiles, 2 cols/cycle) - x only feeds the gradient path
  (the dt*grad term is ~1e-5 of the output scale), so fp8 there is
  harmless. Everything else is fp16: no fp32-HIGH mode anywhere, FWL
  stays available (fp32r movings silently run 4-pass and poison FWL).
- a2 = W2 silu(a1) + c2 is split as (W2/2)a1 + (W2/2)(a1 tanh(a1/2)):
  the (W2/2)a1 term composes with the input layer and streams straight
  from x; the only layer-1 elementwise product is w = a1*tau1 (DVE).
  Tanh and Derivative_silu share one ACT table set
  (derivative_silu_and_others) so there are zero table switches.
- The residual rides the mandatory PSUM->SBUF evacuation (DMA has no
  PSUM route): out = po + x as a DVE tensor_add against a host-prepared
  quadrant-packed fp32 copy of x - full precision, zero extra cost. po
  chunks are quadrant-packed by the l4 matmuls into partition strips
  (512-col chunk h -> partitions 32h..32h+24), halving the evacuation
  free-size on 1024 blocks.
- Queues: x + outputs on Sync (l12 first - it gates the first matmul),
  the merged back-half stationary block on Scalar (one DMA, done before
  the ACT table load), GpSimd entirely unused. Work tiles are allocated
  at uniform width under 6 tags so the Tile context's end-of-kernel
  semaphore drain stays short.
- Pipelining: back half split in two: (d2, l3) issue right after tau(k)
  so d2(k-1) takes the second ACT slot and l3(k-1) does not block the
  w-dependent l2w(k) on the PE queue; (g1, l4, evac, DMA) issue after
  l2w(k). Two small 256-col drain blocks shorten the final serial tail.
  PSUM: a1 double-buffered [2x2 banks], mid arena double-buffered
  [2x2 banks] rotating a2 -> v1 -> po.
"""

import sys

import numpy as np

if "/opt/trn_rl_repo" not in sys.path:
    sys.path.insert(0, "/opt/trn_rl_repo")

import concourse.bass as bass
import concourse.tile as tile
from concourse import mybir

AF = mybir.ActivationFunctionType
F32 = mybir.dt.float32
F16 = mybir.dt.float16
F8 = mybir.dt.float8e4
DR = mybir.MatmulPerfMode.DoubleRow

N_CORES = 8
B, S, N, D = 32, 256, 16, 6
HIDDEN = 32
BLADES = 32
DT = 0.01

TOK_TOTAL = B * S * N          # 131072 positions
TOK_CORE = TOK_TOTAL // N_CORES  # 16384
TPC = 4                        # tokens packed per 128-partition column
GROUPS = TOK_CORE // TPC       # 4096 columns per core
MM = 512                       # matmul free-dim (1 PSUM bank fp32)
BD = 1024                      # max block free-dim (2 PSUM banks)
BLOCKS = [(0, 512), (512, 1024), (1536, 1024), (2560, 1024),
          (3584, 256), (3840, 256)]
OUTW = 512                     # per-block HBM stride of the packed output
NB = len(BLOCKS)

KP = TPC * D                   # 24 partitions of x / out rows
KPI = KP + 1                   # + constant ones row carrying the biases
KT = 13                        # fp8 DoubleRow k-tile height (2*13 >= 26)


def _chunks(wd):
    """512-col matmul chunks of a block: (h, off, width)."""
    out = []
    h = 0
    off = 0
    while off < wd:
        out.append((h, off, min(MM, wd - off)))
        h += 1
        off += MM
    return out


def _build_nc():
    nc = bass.Bass()

    xg8 = nc.dram_tensor("xg8", [KT, 2, GROUPS], F8, kind="ExternalInput")
    xq = nc.dram_tensor("xq", [64, OUTW * NB], F32, kind="ExternalInput")
    l128 = nc.dram_tensor("l128", [KT, 2, 128], F8, kind="ExternalInput")
    cw = nc.dram_tensor("cw", [128, 280], F16, kind="ExternalInput")
    c2b = nc.dram_tensor("c2b", [128, 1], F32, kind="ExternalInput")
    outg = nc.dram_tensor("outg", [64, OUTW * NB], F32, kind="ExternalOutput")

    with tile.TileContext(nc) as tc:
        with (
            tc.tile_pool(name="consts", bufs=1) as consts,
            tc.tile_pool(name="xin", bufs=1) as xin,
            tc.tile_pool(name="work", bufs=3) as work,
            tc.tile_pool(name="psA", bufs=2, space="PSUM") as psA,
            tc.tile_pool(name="psB", bufs=2, space="PSUM") as psB,
        ):
            # l128 gates the very first matmul, so it leads the Sync queue
            # (ahead of the x slabs). The merged back-half stationary block
            # rides Scalar (one DMA, configured before the ACT table load;
            # it is only needed one block in). GpSimd stays fully idle.
            sb_l128 = consts.tile([KT, 2, 128], F8)
            nc.sync.dma_start(out=sb_l128[:], in_=l128[:])
            sb_l1 = sb_l128[:, :, 0:128]
            sb_cw = consts.tile([128, 280], F16)
            nc.scalar.dma_start(out=sb_cw[:], in_=cw[:])
            sb_l2w = sb_cw[:, 0:128]
            sb_l3 = sb_cw[:, 128:256]
            sb_l4 = sb_cw[:, 256:280]
            sb_c2b = consts.tile([128, 1], F32)
            nc.scalar.dma_start(out=sb_c2b[:], in_=c2b[:])

            # x arrives in two slabs (block 0's lands first), the
            # quadrant-packed fp32 residual copy in one.
            W0 = BLOCKS[0][1]
            sb_x8a = xin.tile([KT, 2, W0], F8)
            nc.sync.dma_start(out=sb_x8a[:], in_=xg8[:, :, 0:W0])
            sb_x8b = xin.tile([KT, 2, GROUPS - W0], F8)
            nc.sync.dma_start(out=sb_x8b[:], in_=xg8[:, :, W0:GROUPS])
            sb_xq = xin.tile([64, OUTW * NB], F32)
            nc.sync.dma_start(out=sb_xq[:], in_=xq[:])

            def xslab(bi, lo, hi):
                """SBUF view of x columns [lo, hi) of block bi."""
                c0 = BLOCKS[bi][0]
                if bi == 0:
                    return sb_x8a[:, :, c0 + lo : c0 + hi]
                return sb_x8b[:, :, c0 - W0 + lo : c0 - W0 + hi]

            # Dummy first activation: walrus attaches the ACT table load to
            # the first Activation instruction, which can then carry only a
            # single sync wait. Give it a single-wait warm-up op.
            warm = consts.tile([1, 128], F32)
            nc.vector.memset(warm[:], 0.0)
            nc.scalar.activation(warm[:], warm[:], AF.Derivative_silu)

            def back_early(st):
                """d2 -> v1 for the previous block. Issued right after
                tau(k) so d2(k-1) - whose input has been ready since last
                block - takes the second ACT slot, and l3(k-1) sits ahead
                of the s'-dependent l2(k) on the PE queue. The c2 bias
                rides the ACTIVATE's per-partition bias port (mid holds
                the unbiased a2)."""
                bi, wd, mid, d1 = st
                d2 = work.tile([128, BD], F16, tag="d2")
                nc.scalar.activation(d2[:, 0:wd], mid[:, 0:wd],
                                     AF.Derivative_silu, bias=sb_c2b[:, 0:1])
                for h, off, cw_ in _chunks(wd):
                    ms = slice(off, off + cw_)
                    nc.tensor.matmul(mid[:, ms], sb_l3, d2[:, ms],
                                     start=True, stop=True)
                return d2

            def back_late(st):
                """g1 -> po (quadrant-packed) -> evac(+residual) -> DMA."""
                bi, wd, mid, d1 = st
                g1 = work.tile([128, BD], F16, tag="g1")
                nc.vector.tensor_mul(g1[:, 0:wd], mid[:, 0:wd], d1[:, 0:wd])

                # po = blockdiag(Bout) @ g1, chunk h on partitions
                # 32h..32h+24 (overwrites v1, dead after g1).
                nch = 0
                for h, off, cw_ in _chunks(wd):
                    po = mid[32 * h : 32 * h + KP, 0:cw_]
                    nc.tensor.matmul(po, sb_l4, g1[:, off : off + cw_],
                                     start=True, stop=True)
                    nch += 1

                # Mandatory PSUM->SBUF evacuation doubles as the residual
                # add against the quadrant-packed fp32 x.
                ew = min(wd, MM)
                sb_o = work.tile([64, MM], F32, tag="o")
                nc.vector.tensor_add(
                    sb_o[0 : 32 * nch, 0:ew],
                    mid[0 : 32 * nch, 0:ew],
                    sb_xq[0 : 32 * nch, OUTW * bi : OUTW * bi + ew],
                )
                nc.sync.dma_start(
                    out=outg[0 : 32 * nch, OUTW * bi : OUTW * bi + ew],
                    in_=sb_o[0 : 32 * nch, 0:ew],
                )

            pending = None
            for bi, (c0, wd) in enumerate(BLOCKS):
                # a1 = blockdiag(A) @ x + c1 (c1 rides the ones row);
                # fp8 DoubleRow: contraction over 2 k-tiles of 13 rows.
                a1 = psA.tile([128, BD], F32, tag="a1")
                for h, off, cw_ in _chunks(wd):
                    nc.tensor.matmul(a1[:, off : off + cw_], sb_l1,
                                     xslab(bi, off, off + cw_),
                                     start=True, stop=True, perf_mode=DR)

                # tau = tanh(0.5*a1)
                tau = work.tile([128, BD], F16, tag="tau")
                nc.scalar.activation(tau[:, 0:wd], a1[:, 0:wd], AF.Tanh,
                                     scale=0.5)

                # s' = (tau + 1) * a1 = 2*silu(a1), fused on the DVE
                # (scalar_tensor_tensor: out = (in0 op0 scalar) op1 in1).
                # This folds what used to be a separate x-borne matmul pass
                # ((W2/2) a1 streamed from x) into the existing elementwise
                # slot: a2 = (W2/2) s' in a single PE pass. Issued in
                # 512-col chunks so each l2 matmul starts as soon as its
                # half of s' lands (the tau->s'->l2->d2 chain is the
                # latency-critical loop of the block pipeline).
                w = work.tile([128, BD], F16, tag="w")
                for h, off, cw_ in _chunks(wd):
                    ms = slice(off, off + cw_)
                    nc.vector.scalar_tensor_tensor(
                        w[:, ms], tau[:, ms], 1.0, a1[:, ms],
                        op0=mybir.AluOpType.add, op1=mybir.AluOpType.mult,
                    )

                if pending is not None:
                    back_early(pending)

                # d1 = silu'(a1) (consumer g1 is a block away, so d2(k-1)
                # above takes the second ACT slot)
                d1 = work.tile([128, BD], F16, tag="d1")
                nc.scalar.activation(d1[:, 0:wd], a1[:, 0:wd],
                                     AF.Derivative_silu)

                # a2 (sans c2, added at the d2 ACTIVATE) = blockdiag(W2/2) @ s'
                mid = psB.tile([128, BD], F32, tag="mid")
                for h, off, cw_ in _chunks(wd):
                    ms = slice(off, off + cw_)
                    nc.tensor.matmul(mid[:, ms], sb_l2w, w[:, ms],
                                     start=True, stop=True)

                if pending is not None:
                    back_late(pending)

                pending = (bi, wd, mid, d1)

            back_early(pending)
            back_late(pending)

    return nc


def _split_multi_waits(nc):
    """This walrus build rejects engine instructions carrying more than one
    sync wait ("Too many sync wait commands"). Hoist all but one wait of
    each instruction onto standalone NoOps issued just before it on the
    same engine (engines execute their queue in order, so semantics are
    preserved)."""
    for f in nc.m.functions:
        for b in f.blocks:
            insts = list(b.instructions)
            out = []
            changed = False
            for inst in insts:
                # This walrus build also rejects the raw-ISA
                # EVENT_SEMAPHORE_RANGE_CLEAR Tile emits at context end
                # ("ISA wrong length" - ISA table version skew). The NEFF
                # preamble re-initializes semaphores, so drop it.
                if (
                    type(inst).__name__ == "InstISA"
                    and getattr(inst, "op_name", "") == "EVENT_SEMAPHORE_RANGE_CLEAR"
                ):
                    changed = True
                    continue
                si = getattr(inst, "sync_info", None)
                waits = list(si.on_wait) if si is not None and si.on_wait else []
                if len(waits) > 1:
                    changed = True
                    for k, w in enumerate(waits[:-1]):
                        nop = mybir.InstNoOp(name=f"{inst.name}-w{k}", ins=[], outs=[])
                        nop.engine = inst.engine
                        nop.sync_info = mybir.SyncInfo(on_wait=[w], on_update=[])
                        out.append(nop)
                    inst.sync_info = mybir.SyncInfo(
                        on_wait=[waits[-1]], on_update=list(si.on_update or [])
                    )
                out.append(inst)
            if changed:
                b.instructions = out
    return nc


_NC_CACHE = None


def _get_nc():
    global _NC_CACHE
    if _NC_CACHE is None:
        _NC_CACHE = _split_multi_waits(_build_nc())
    return _NC_CACHE


def _fold_kt(rows):
    """[26, M] -> [13, 2, M] DoubleRow k-tile layout (row = 13*ko + ki)."""
    return rows.reshape(2, KT, -1).transpose(1, 0, 2)


def _prep_weights(W_in, b_in, W1, b1, W2, b2, W3, b3):
    """Host-side constant folding into the kernel's stationary layouts."""
    import ml_dtypes

    F8NP = ml_dtypes.float8_e4m3

    W_in = np.asarray(W_in, np.float64)
    b_in = np.asarray(b_in, np.float64)
    W1 = np.asarray(W1, np.float64)
    b1 = np.asarray(b1, np.float64)
    W2 = np.asarray(W2, np.float64)
    b2 = np.asarray(b2, np.float64)
    W3 = np.asarray(W3, np.float64)

    Win0 = W_in[:, ::BLADES]            # [6, 8]
    bin0 = b_in[::BLADES]               # [8]
    A = W1 @ Win0.T                     # [32, 6]
    c1 = W1 @ bin0 + b1[:, 0]           # [32]
    c2 = b2[:, 0]                       # [32]
    w3 = W3[0, :]                       # [32]

    # Bout[d, c]: out[d] += dt*dx[d+3] (d<3), -dt*dx[d-3] (d>=3); dx = A^T g1
    Bout = np.zeros((D, HIDDEN))
    Bout[0:3, :] = DT * A[:, 3:6].T
    Bout[3:6, :] = -DT * A[:, 0:3].T

    # a2 = W2 silu(a1) + c2 = (W2/2) s' + c2 with s' = (1 + tanh(a1/2)) a1
    # = 2 silu(a1); c2 is applied through the d2 ACTIVATE's bias port.
    l1f = np.zeros((2 * KT, 128), np.float32)
    cwm = np.zeros((128, 280), np.float16)
    l2wm = cwm[:, 0:128]
    l3 = cwm[:, 128:256]
    l4 = cwm[:, 256:280]
    c2b = np.zeros((128, 1), np.float32)
    for tl in range(TPC):
        # l1[6tl+d, 32tl+c] = A[c, d]; l1[24, 32tl+c] = c1[c]
        l1f[6 * tl : 6 * tl + 6, 32 * tl : 32 * tl + 32] = A.T
        l1f[KP, 32 * tl : 32 * tl + 32] = c1
        # l2w[32tl+ci, 32tl+co] = W2[co, ci] / 2
        l2wm[32 * tl : 32 * tl + 32, 32 * tl : 32 * tl + 32] = (
            0.5 * W2.T
        ).astype(np.float16)
        # l3[32tl+co, 32tl+ci] = w3[co] * W2[co, ci]
        l3[32 * tl : 32 * tl + 32, 32 * tl : 32 * tl + 32] = (
            w3[:, None] * W2
        ).astype(np.float16)
        # l4[32tl+c, 6tl+d] = Bout[d, c]
        l4[32 * tl : 32 * tl + 32, 6 * tl : 6 * tl + 6] = Bout.T.astype(
            np.float16
        )
        c2b[32 * tl : 32 * tl + 32, 0] = c2

    return {
        "l128": np.ascontiguousarray(_fold_kt(l1f)).astype(F8NP),
        "cw": cwm,
        "c2b": c2b,
    }


def _shard_x(x):
    """[B,S,N,D] -> per-core (fp8 [13, 2, GROUPS] DoubleRow matmul layout
    with ones row, fp32 [64, OUTW*NB] quadrant-packed residual layout)."""
    import ml_dtypes

    F8NP = ml_dtypes.float8_e4m3

    xf = np.ascontiguousarray(np.asarray(x, np.float32)).reshape(TOK_TOTAL, D)
    shards = []
    for c in range(N_CORES):
        xc = xf[c * TOK_CORE : (c + 1) * TOK_CORE]          # [16384, 6]
        xp = xc.reshape(GROUPS, TPC, D).transpose(1, 2, 0).reshape(KP, GROUPS)
        xe = np.zeros((2 * KT, GROUPS), np.float32)
        xe[:KP] = xp
        xe[KP] = 1.0
        x8 = np.ascontiguousarray(_fold_kt(xe)).astype(F8NP)
        xqc = np.zeros((64, OUTW * NB), np.float32)
        for bi, (c0, wd) in enumerate(BLOCKS):
            for h, off, cw_ in _chunks(wd):
                xqc[32 * h : 32 * h + KP, OUTW * bi : OUTW * bi + cw_] = xp[
                    :, c0 + off : c0 + off + cw_
                ]
        shards.append((x8, xqc))
    return shards


def _unshard_out(outs):
    """list of per-core [64, OUTW*NB] -> [B,S,N,D].

    Block bi covers global cols c0..c0+wd; its 512-col chunk h sits on
    partitions 32h..32h+24 of outg[:, OUTW*bi : OUTW*bi+cw]."""
    full = np.empty((TOK_TOTAL, D), np.float32)
    for c, og in enumerate(outs):
        og = np.asarray(og)
        oc = np.empty((KP, GROUPS), np.float32)
        for bi, (c0, wd) in enumerate(BLOCKS):
            for h, off, cw_ in _chunks(wd):
                oc[:, c0 + off : c0 + off + cw_] = og[
                    32 * h : 32 * h + KP, OUTW * bi : OUTW * bi + cw_
                ]
        occ = oc.reshape(TPC, D, GROUPS).transpose(2, 0, 1).reshape(TOK_CORE, D)
        full[c * TOK_CORE : (c + 1) * TOK_CORE] = occ
    return full.reshape(B, S, N, D)


# Test-harness knobs (ignored in normal use): set kernel._TRACE = True to
# collect an NTFF profile; the BassKernelResults lands in kernel._LAST_RES.
_TRACE = False
_LAST_RES = None


def kernel(x, W_in, b_in, W1, b1, W2, b2, W3, b3):
    global _LAST_RES
    from concourse.bass_utils import run_bass_kernel_spmd

    nc = _get_nc()
    consts = _prep_weights(W_in, b_in, W1, b1, W2, b2, W3, b3)
    shards = _shard_x(x)
    in_maps = [
        {"xg8": shards[c][0], "xq": shards[c][1], **consts}
        for c in range(N_CORES)
    ]
    res = run_bass_kernel_spmd(nc, in_maps, list(range(N_CORES)), trace=_TRACE)
    _LAST_RES = res
    return _unshard_out([res.results[c]["outg"] for c in range(N_CORES)])
